# revision 2
# baseline (speedup 1.0000x reference)
"""Trainium2 Bass kernel for nn_CustomCNNLayer_84559316124470.

The reference computes, per batch b:
    win[b,c,s,m]   = xp[b,c,s+m]                    (xp = x padded with K-1 zeros)
    xw[b,c,s,m,l]  = win[b,c,s,m] * stft_w[l,m]
    xr             = xw.reshape(b, c*K*NK, s)       (raw row-major reshape)
    out            = relu(conv_w @ xr + bias)       (1x1 conv over channels)

Because K*NK == S/2 == 2048, the raw reshape maps
    xr[b, c*2048 + q, p*2048 + m*32 + l] = xp[b, c, 2q+p+m] * stft_w[l, m]
(with s = 2q+p). Hence, with h[b,o,r] = sum_{c,q} conv_w[o, c*2048+q] * xp[b,c,2q+r]
(r in [0, 65)):
    out[b, o, p*2048 + m*32 + l] = relu(stft_w[l,m] * h[b,o,p+m] + bias[o])

So the 8.6 GMAC/batch dense matmul collapses to a (512x4096)@(4096x65)
strided correlation plus a sparse expansion matmul:
    out_block(pair,p) = hT(p:p+64).T @ W + bias,  W[m, m*32+l] = stft_w[l, m]

Sharding: output channels o are split across the 8 cores (64 rows each);
x-derived windows and the expansion matrix are replicated. No collectives.
"""

import numpy as np

import concourse.bass as bass
import concourse.tile as tile
from concourse import bacc, mybir
from concourse.bass_utils import run_bass_kernel_spmd

B, C, S = 4, 2, 4096
K, NK, OUT = 64, 32, 512
Q = K * NK            # 2048 == S // 2
R = K + 1             # 65 shift taps
NCORES = 8
OSH = OUT // NCORES   # 64 output channels per core
KT = 32               # contraction tiles of 128 over c*Q = 4096
W260 = B * R          # mm1 moving free dim: 4 batches x 65 taps
F32 = mybir.dt.float32

_PROGRAM = None


def _kernel_body(tc, out, xw, cwt, wexp, bias2, ident):
    nc = tc.nc
    from contextlib import ExitStack

    with ExitStack() as ctx:
        const = ctx.enter_context(tc.tile_pool(name="const", bufs=1))
        psum_h = ctx.enter_context(tc.tile_pool(name="psum_h", bufs=1, space="PSUM"))
        psum_t = ctx.enter_context(tc.tile_pool(name="psum_t", bufs=2, space="PSUM"))
        psum_o = ctx.enter_context(tc.tile_pool(name="psum_o", bufs=4, space="PSUM"))
        sbuf_o = ctx.enter_context(tc.tile_pool(name="sbuf_o", bufs=4))

        b_sb = const.tile([128, 1], F32, tag="b_sb")
        id_sb = const.tile([K, K], F32, tag="id_sb")
        W_sb = const.tile([K, Q], F32, tag="W_sb")
        nc.sync.dma_start(b_sb[:], bias2)
        nc.sync.dma_start(id_sb[:], ident)

        # Chunked weight/window loads so mm1 can start before all input lands.
        NCH = 4
        KTC = KT // NCH  # 8 contraction tiles per chunk
        cw_tiles = []
        X_tiles = []
        for ch in range(NCH):
            cw_t = const.tile([128, KTC * OSH], F32, tag=f"cw{ch}")
            nc.sync.dma_start(cw_t[:], cwt[:, bass.ts(ch, KTC * OSH)])
            X_t = const.tile([128, KTC * W260], F32, tag=f"X{ch}")
            nc.sync.dma_start(X_t[:], xw[:, bass.ts(ch, KTC * W260)])
            cw_tiles.append(cw_t)
            X_tiles.append(X_t)
            if ch == 1:
                nc.sync.dma_start(W_sb[:], wexp)

        # mm1: h[o', b*65 + r] = sum_g conv_w[o_shard+o', g] * X_b[g, r]
        h_ps = psum_h.tile([OSH, W260], F32, tag="h_ps")
        for ch in range(NCH):
            for kt in range(KTC):
                nc.tensor.matmul(
                    h_ps[:],
                    cw_tiles[ch][:, bass.ts(kt, OSH)],
                    X_tiles[ch][:, bass.ts(kt, W260)],
                    start=(ch == 0 and kt == 0),
                    stop=(ch == NCH - 1 and kt == KTC - 1),
                )
        h_sb = const.tile([OSH, W260], F32, tag="h_sb")
        nc.vector.tensor_copy(h_sb[:], h_ps[:])

        # Transpose h slices: hT[pair][p][m, z*64+o'] = h[o', (2*pair+z)*65 + p + m]
        hT = [[None, None], [None, None]]
        for pair in range(2):
            for p in range(2):
                hT_sb = const.tile([K, 2 * OSH], F32, tag=f"hT{pair}{p}")
                hT[pair][p] = hT_sb
        for b in range(B):
            for p in range(2):
                t_ps = psum_t.tile([K, OSH], F32, tag="t_ps")
                nc.tensor.transpose(
                    t_ps[:], h_sb[:, b * R + p : b * R + p + K], id_sb[:]
                )
                nc.vector.tensor_copy(
                    hT[b // 2][p][:, bass.ts(b % 2, OSH)], t_ps[:]
                )

        # mm2 + bias/ReLU + store: out rows (z*64+o') for batches (2*pair+z)
        for pair in range(2):
            for p in range(2):
                for n in range(4):
                    o_ps = psum_o.tile([128, 512], F32, tag="o_ps")
                    nc.tensor.matmul(
                        o_ps[:],
                        hT[pair][p][:],
                        W_sb[:, bass.ts(n, 512)],
                        start=True,
                        stop=True,
                    )
                    o_sb = sbuf_o.tile([128, 512], F32, tag="o_sb")
                    nc.scalar.activation(
                        o_sb[:], o_ps[:], mybir.ActivationFunctionType.Relu,
                        bias=b_sb[:],
                    )
                    nc.sync.dma_start(
                        out[
                            pair * 128 : (pair + 1) * 128,
                            p * Q + n * 512 : p * Q + (n + 1) * 512,
                        ],
                        o_sb[:],
                    )


def _build_program():
    nc = bacc.Bacc(
        "TRN2", target_bir_lowering=False, debug=False, num_devices=NCORES
    )
    xw = nc.dram_tensor("xw", [128, KT * W260], F32, kind="ExternalInput").ap()
    cwt = nc.dram_tensor("cwt", [128, KT * OSH], F32, kind="ExternalInput").ap()
    wexp = nc.dram_tensor("wexp", [K, Q], F32, kind="ExternalInput").ap()
    bias2 = nc.dram_tensor("bias2", [128, 1], F32, kind="ExternalInput").ap()
    ident = nc.dram_tensor("ident", [K, K], F32, kind="ExternalInput").ap()
    out = nc.dram_tensor("out", [2 * 128, S], F32, kind="ExternalOutput").ap()

    with tile.TileContext(nc) as tc:
        _kernel_body(tc, out, xw, cwt, wexp, bias2, ident)
    nc.compile()
    return nc


def _host_prepare(x, stft_w, conv_w, conv_b):
    """Build per-core input maps (all float32 numpy)."""
    x = np.ascontiguousarray(x, dtype=np.float32)
    xp = np.zeros((B, C, 2 * Q + K), dtype=np.float32)  # padded to 4160
    xp[:, :, :S] = x
    # windows[b, c, q, r] = xp[b, c, 2q + r]
    sb, sc, ss = xp.strides
    win = np.lib.stride_tricks.as_strided(
        xp, shape=(B, C, Q, R), strides=(sb, sc, 2 * ss, ss)
    )
    # X_b[g, r] with g = c*Q + q; tile g = kt*128 + p; layout (p, kt, b, r)
    Xf = win.reshape(B, C * Q, R)                       # (4, 4096, 65)
    xw_host = np.ascontiguousarray(
        Xf.reshape(B, KT, 128, R).transpose(2, 1, 0, 3)  # (128, 32, 4, 65)
    ).reshape(128, KT * W260)

    # Expansion matrix W[m, m*32 + l] = stft_w[l, m]
    wexp = np.zeros((K, Q), dtype=np.float32)
    for m in range(K):
        wexp[m, m * NK : (m + 1) * NK] = stft_w[:, m]

    ident = np.eye(K, dtype=np.float32)

    in_maps = []
    for i in range(NCORES):
        cw_sh = conv_w[i * OSH : (i + 1) * OSH, :]      # (64, 4096)
        # cwt[p, kt*64 + m] = conv_w[i*64 + m, kt*128 + p]
        cwt = np.ascontiguousarray(
            cw_sh.reshape(OSH, KT, 128).transpose(2, 1, 0)  # (128, 32, 64)
        ).reshape(128, KT * OSH)
        bias2 = np.tile(conv_b[i * OSH : (i + 1) * OSH], 2).reshape(128, 1)
        in_maps.append(
            {
                "xw": xw_host,
                "cwt": cwt,
                "wexp": wexp,
                "bias2": np.ascontiguousarray(bias2, dtype=np.float32),
                "ident": ident,
            }
        )
    return in_maps


_LAST_RESULTS = None


def kernel(x, stft_w, conv_w, conv_b):
    global _PROGRAM, _LAST_RESULTS
    if _PROGRAM is None:
        _PROGRAM = _build_program()
    in_maps = _host_prepare(
        np.asarray(x), np.asarray(stft_w, dtype=np.float32),
        np.asarray(conv_w, dtype=np.float32), np.asarray(conv_b, dtype=np.float32),
    )
    res = run_bass_kernel_spmd(_PROGRAM, in_maps, list(range(NCORES)))
    _LAST_RESULTS = res
    # per-core out: (256, 4096); rows pair*128 + z*64 + o' -> (b=2*pair+z, o=i*64+o')
    full = np.empty((B, OUT, S), dtype=np.float32)
    for i in range(NCORES):
        full[:, i * OSH : (i + 1) * OSH, :] = res.results[i]["out"].reshape(
            B, OSH, S
        )
    return full


if __name__ == "__main__":
    rng = np.random.default_rng(0)
    out = kernel(
        rng.standard_normal((B, C, S), dtype=np.float32),
        rng.standard_normal((NK, K), dtype=np.float32),
        (rng.standard_normal((OUT, C * K * NK)) * 0.02).astype(np.float32),
        (rng.standard_normal((OUT,)) * 0.02).astype(np.float32),
    )
    print(out.shape, out.dtype, float(np.abs(out).max()))


# revision 4
# speedup vs baseline: 1.1933x; 1.1933x over previous
"""Trainium2 Bass kernel for nn_CustomCNNLayer_84559316124470.

The reference computes, per batch b:
    win[b,c,s,m]   = xp[b,c,s+m]                    (xp = x padded with K-1 zeros)
    xw[b,c,s,m,l]  = win[b,c,s,m] * stft_w[l,m]
    xr             = xw.reshape(b, c*K*NK, s)       (raw row-major reshape)
    out            = relu(conv_w @ xr + bias)       (1x1 conv over channels)

Because K*NK == S/2 == 2048, the raw reshape maps
    xr[b, c*2048 + q, p*2048 + m*32 + l] = xp[b, c, 2q+p+m] * stft_w[l, m]
(with s = 2q+p). Hence, with h[b,o,r] = sum_{c,q} conv_w[o, c*2048+q] * xp[b,c,2q+r]
(r in [0, 65)):
    out[b, o, p*2048 + m*32 + l] = relu(stft_w[l,m] * h[b,o,p+m] + bias[o])

So the dense 8.6 GMAC/batch matmul collapses to a (512x4096)@(4096x65)
strided correlation (tensor engine) plus a per-element broadcast expansion
(vector engine) and bias+ReLU (scalar engine).

Sharding: output channels o split across the 8 cores (64 rows each);
window matrices replicated. No collectives.

Precision: mm1 runs on the PE in bf16. With PASSES=3 the fp32 operands are
split hi/lo into bf16 pairs and three accumulating matmuls recover ~fp32
accuracy (error ~1e-5 rel. vs ~1e-3 for PASSES=1); fp32 PE matmuls run in
6-pass LOW_HIGH mode and are not competitive.
"""

import numpy as np
import ml_dtypes

import concourse.bass as bass
import concourse.tile as tile
from concourse import bacc, mybir
from concourse.bass_utils import run_bass_kernel_spmd

B, C, S = 4, 2, 4096
K, NK, OUT = 64, 32, 512
Q = K * NK            # 2048 == S // 2
R = K + 1             # 65 shift taps
NCORES = 8
OSH = OUT // NCORES   # 64 output channels per core
KT = 32               # contraction tiles of 128 over c*Q = 4096
W260 = B * R          # per-kt rhs free dim: [z=0 | z=1] x [pair 0 | pair 1] x r
PASSES = 3            # 1 = plain bf16, 3 = hi/lo split (near-fp32)
NCH = 4               # DMA chunks over kt
KTC = KT // NCH
F32 = mybir.dt.float32
BF16 = mybir.dt.bfloat16

_PROGRAM = None
_LAST_RESULTS = None


def _kernel_body(tc, out, ins):
    nc = tc.nc
    from contextlib import ExitStack

    with ExitStack() as ctx:
        const = ctx.enter_context(tc.tile_pool(name="const", bufs=1))
        psum_h = ctx.enter_context(tc.tile_pool(name="psum_h", bufs=1, space="PSUM"))
        tmp_p = ctx.enter_context(tc.tile_pool(name="tmp_p", bufs=2))
        sbuf_o = ctx.enter_context(tc.tile_pool(name="sbuf_o", bufs=3))

        b_sb = const.tile([128, 1], F32, tag="b_sb")
        nc.sync.dma_start(b_sb[:], ins["bias2"])
        T_sb = const.tile([128, Q], F32, tag="T_sb")
        nc.sync.dma_start(T_sb[:], ins["trow"].to_broadcast((128, Q)))

        # weight/window pairs per pass: (conv lhsT, window rhs)
        pass_srcs = [("ch", "xh")] if PASSES == 1 else [
            ("ch", "xh"), ("ch", "xl"), ("cl", "xh")
        ]
        names = (
            ["ch", "xh"] if PASSES == 1 else ["ch", "cl", "xh", "xl"]
        )
        # chunked loads so mm1 can start early; cw chunk then X chunk
        tiles = {n: [] for n in names}
        for chk in range(NCH):
            for n in names:
                width = KTC * (OSH if n[0] == "c" else W260)
                t = const.tile([128, width], BF16, tag=f"{n}{chk}")
                nc.sync.dma_start(t[:], ins[n][:, bass.ts(chk, width)])
                tiles[n].append(t)

        # mm1: h3[z*64+o', pr*65+r] = sum_g conv_w[o_shard+o', g] * X_{2pr+z}[g, r]
        # z kept outer: interleaved start= groups on partition halves trip the
        # simulator's PSUM pending-zero model.
        h3 = psum_h.tile([128, 2 * R], F32, tag="h3")
        n_mm = NCH * KTC * len(pass_srcs)
        for z in range(2):
            i_mm = 0
            for chk in range(NCH):
                for kt in range(KTC):
                    for cw_n, x_n in pass_srcs:
                        nc.tensor.matmul(
                            h3[z * OSH : (z + 1) * OSH, :],
                            tiles[cw_n][chk][:, bass.ts(kt, OSH)],
                            tiles[x_n][chk][
                                :,
                                kt * W260 + z * 2 * R : kt * W260 + (z + 1) * 2 * R,
                            ],
                            start=(i_mm == 0),
                            stop=(i_mm == n_mm - 1),
                            skip_group_check=True,
                        )
                        i_mm += 1

        # expansion: out[z*64+o', u] = relu(h3[z*64+o', pr*65+p+u//32]*T[u] + bias)
        for pr in range(2):
            for p in range(2):
                off = pr * R + p
                h_exp = h3[:, off : off + K].unsqueeze(2).to_broadcast((128, K, NK))
                tmp = tmp_p.tile([128, Q], F32, tag="tmp")
                nc.vector.tensor_tensor(
                    tmp.rearrange("a (m l) -> a m l", l=NK),
                    h_exp,
                    T_sb.rearrange("a (m l) -> a m l", l=NK),
                    mybir.AluOpType.mult,
                )
                o_sb = sbuf_o.tile([128, Q], F32, tag="o_sb")
                nc.scalar.activation(
                    o_sb[:], tmp[:], mybir.ActivationFunctionType.Relu, bias=b_sb[:]
                )
                nc.sync.dma_start(
                    out[pr * 128 : (pr + 1) * 128, p * Q : (p + 1) * Q], o_sb[:]
                )


def _build_program():
    nc = bacc.Bacc(
        "TRN2", target_bir_lowering=False, debug=False, num_devices=NCORES
    )
    ins = {}
    ins["xh"] = nc.dram_tensor("xh", [128, KT * W260], BF16, kind="ExternalInput").ap()
    ins["ch"] = nc.dram_tensor("ch", [128, KT * OSH], BF16, kind="ExternalInput").ap()
    if PASSES == 3:
        ins["xl"] = nc.dram_tensor(
            "xl", [128, KT * W260], BF16, kind="ExternalInput"
        ).ap()
        ins["cl"] = nc.dram_tensor(
            "cl", [128, KT * OSH], BF16, kind="ExternalInput"
        ).ap()
    ins["trow"] = nc.dram_tensor("trow", [1, Q], F32, kind="ExternalInput").ap()
    ins["bias2"] = nc.dram_tensor("bias2", [128, 1], F32, kind="ExternalInput").ap()
    out = nc.dram_tensor("out", [2 * 128, S], F32, kind="ExternalOutput").ap()

    with tile.TileContext(nc) as tc:
        _kernel_body(tc, out, ins)
    nc.compile()
    return nc


def _split_bf16(a):
    hi = a.astype(ml_dtypes.bfloat16)
    lo = (a - hi.astype(np.float32)).astype(ml_dtypes.bfloat16)
    return hi, lo


def _host_prepare(x, stft_w, conv_w, conv_b):
    """Build per-core input maps."""
    x = np.ascontiguousarray(x, dtype=np.float32)
    xp = np.zeros((B, C, 2 * Q + K), dtype=np.float32)  # padded to 4160
    xp[:, :, :S] = x
    sb_, sc_, ss_ = xp.strides
    win = np.lib.stride_tricks.as_strided(
        xp, shape=(B, C, Q, R), strides=(sb_, sc_, 2 * ss_, ss_)
    )
    Xf = win.reshape(B, C * Q, R)                      # (4, 4096, 65), b=2*pr+z
    # layout [p, kt, z, pr, r]: batch order (z,pr) -> b = [0, 2, 1, 3]
    X5 = np.ascontiguousarray(
        Xf[[0, 2, 1, 3]].reshape(2, 2, KT, 128, R).transpose(3, 2, 0, 1, 4)
    ).reshape(128, KT * W260)
    xh, xl = _split_bf16(X5)

    trow = np.zeros((1, Q), dtype=np.float32)
    trow[0] = np.ascontiguousarray(stft_w.T).reshape(Q)  # T[m*32+l] = stft_w[l, m]

    in_maps = []
    for i in range(NCORES):
        cw_sh = conv_w[i * OSH : (i + 1) * OSH, :]     # (64, 4096)
        cwt = np.ascontiguousarray(
            cw_sh.reshape(OSH, KT, 128).transpose(2, 1, 0)  # (128, 32, 64)
        ).reshape(128, KT * OSH)
        ch, cl = _split_bf16(cwt)
        bias2 = np.ascontiguousarray(
            np.tile(conv_b[i * OSH : (i + 1) * OSH], 2).reshape(128, 1),
            dtype=np.float32,
        )
        m = {"xh": xh, "ch": ch, "trow": trow, "bias2": bias2}
        if PASSES == 3:
            m["xl"] = xl
            m["cl"] = cl
        in_maps.append(m)
    return in_maps


def kernel(x, stft_w, conv_w, conv_b):
    global _PROGRAM, _LAST_RESULTS
    if _PROGRAM is None:
        _PROGRAM = _build_program()
    in_maps = _host_prepare(
        np.asarray(x), np.asarray(stft_w, dtype=np.float32),
        np.asarray(conv_w, dtype=np.float32), np.asarray(conv_b, dtype=np.float32),
    )
    res = run_bass_kernel_spmd(_PROGRAM, in_maps, list(range(NCORES)))
    _LAST_RESULTS = res
    # per-core out: (256, 4096); rows pr*128 + z*64 + o' -> (b=2*pr+z, o=i*64+o')
    full = np.empty((B, OUT, S), dtype=np.float32)
    for i in range(NCORES):
        full[:, i * OSH : (i + 1) * OSH, :] = res.results[i]["out"].reshape(
            B, OSH, S
        )
    return full


if __name__ == "__main__":
    rng = np.random.default_rng(0)
    out = kernel(
        rng.standard_normal((B, C, S), dtype=np.float32),
        rng.standard_normal((NK, K), dtype=np.float32),
        (rng.standard_normal((OUT, C * K * NK)) * 0.02).astype(np.float32),
        (rng.standard_normal((OUT,)) * 0.02).astype(np.float32),
    )
    print(out.shape, out.dtype, float(np.abs(out).max()))


# revision 5
# speedup vs baseline: 1.2255x; 1.0269x over previous
"""Trainium2 Bass kernel for nn_CustomCNNLayer_84559316124470.

The reference computes, per batch b:
    win[b,c,s,m]   = xp[b,c,s+m]                    (xp = x padded with K-1 zeros)
    xw[b,c,s,m,l]  = win[b,c,s,m] * stft_w[l,m]
    xr             = xw.reshape(b, c*K*NK, s)       (raw row-major reshape)
    out            = relu(conv_w @ xr + bias)       (1x1 conv over channels)

Because K*NK == S/2 == 2048, the raw reshape maps
    xr[b, c*2048 + q, p*2048 + m*32 + l] = xp[b, c, 2q+p+m] * stft_w[l, m]
(with s = 2q+p). Hence, with h[b,o,r] = sum_{c,q} conv_w[o, c*2048+q] * xp[b,c,2q+r]
(r in [0, 65)):
    out[b, o, p*2048 + m*32 + l] = relu(stft_w[l,m] * h[b,o,p+m] + bias[o])

So the dense 8.6 GMAC/batch matmul collapses to a (512x4096)@(4096x65)
strided correlation (tensor engine) plus a per-element broadcast expansion
(vector engine) and bias+ReLU (scalar/vector engines).

Sharding: output channels o split across the 8 cores (64 rows each);
window matrices replicated. No collectives.

Precision: mm1 runs on the PE in bf16. With PASSES=3 the fp32 operands are
split hi/lo into bf16 pairs and three accumulating matmuls recover ~fp32
accuracy (error ~1e-5 rel. vs ~1e-3 for PASSES=1); fp32 PE matmuls run in
multi-pass LOW_HIGH mode and are not competitive.
"""

import numpy as np
import ml_dtypes

import concourse.bass as bass
import concourse.tile as tile
from concourse import bacc, mybir
from concourse.bass_utils import run_bass_kernel_spmd

B, C, S = 4, 2, 4096
K, NK, OUT = 64, 32, 512
Q = K * NK            # 2048 == S // 2
R = K + 1             # 65 shift taps
NCORES = 8
OSH = OUT // NCORES   # 64 output channels per core
KT = 32               # contraction tiles of 128 over c*Q = 4096
W260 = B * R          # per-kt rhs free dim: [z=0 | z=1] x [pair 0 | pair 1] x r
PASSES = 3            # 1 = plain bf16, 3 = hi/lo split (near-fp32)
NCH = 4               # DMA chunks over kt
KTC = KT // NCH
NPASS_COLS = (2 * OSH + 2 * W260) if PASSES == 3 else (OSH + W260)
CHUNK_W = KTC * NPASS_COLS  # columns per chunk in the merged bf16 buffer
F32 = mybir.dt.float32
BF16 = mybir.dt.bfloat16

_PROGRAM = None
_LAST_RESULTS = None


def _kernel_body(tc, out, ins):
    nc = tc.nc
    from contextlib import ExitStack

    with ExitStack() as ctx:
        const = ctx.enter_context(tc.tile_pool(name="const", bufs=1))
        psum_h = ctx.enter_context(tc.tile_pool(name="psum_h", bufs=1, space="PSUM"))
        tmp_p = ctx.enter_context(tc.tile_pool(name="tmp_p", bufs=2))
        sbuf_o = ctx.enter_context(tc.tile_pool(name="sbuf_o", bufs=3))

        # chunk layout (bf16): [ch (KTC*64) | cl | xh (KTC*260) | xl]
        co, xo = 0, (2 if PASSES == 3 else 1) * KTC * OSH
        cw_names = ["ch", "cl"][: 1 if PASSES == 1 else 2]
        chunks = []
        for chk in range(NCH):
            w_t = const.tile([128, CHUNK_W], BF16, tag=f"w{chk}")
            nc.sync.dma_start(w_t[:], ins["wbuf"][:, bass.ts(chk, CHUNK_W)])
            chunks.append(w_t)
            if chk == 1:
                T_sb = const.tile([128, Q], F32, tag="T_sb")
                nc.sync.dma_start(T_sb[:], ins["trow"].to_broadcast((128, Q)))
                b_sb = const.tile([128, 1], F32, tag="b_sb")
                nc.sync.dma_start(b_sb[:], ins["bias2"])

        # mm1: h[o', z*130 + pr*65 + r] = sum_g conv_w[o_shard+o', g]*X_{2pr+z}[g,r]
        pass_offs = [(co, xo)] if PASSES == 1 else [
            (co, xo),                      # ch @ xh
            (co, xo + KTC * W260),         # ch @ xl
            (co + KTC * OSH, xo),          # cl @ xh
        ]
        h_ps = psum_h.tile([OSH, W260], F32, tag="h_ps")
        n_mm = NCH * KTC * len(pass_offs)
        i_mm = 0
        for chk in range(NCH):
            for kt in range(KTC):
                for c_off, x_off in pass_offs:
                    nc.tensor.matmul(
                        h_ps[:],
                        chunks[chk][:, c_off + kt * OSH : c_off + (kt + 1) * OSH],
                        chunks[chk][:, x_off + kt * W260 : x_off + (kt + 1) * W260],
                        start=(i_mm == 0),
                        stop=(i_mm == n_mm - 1),
                    )
                    i_mm += 1

        # redistribute h (64, [z|pr|r]) -> h2 (z*64+o', pr*65+r) via sb2sb DMA
        h_sb = const.tile([OSH, W260], F32, tag="h_sb")
        nc.vector.tensor_copy(h_sb[:], h_ps[:])
        h2_sb = const.tile([128, 2 * R], F32, tag="h2_sb")
        for z in range(2):
            nc.sync.dma_start(
                h2_sb[z * OSH : (z + 1) * OSH, :],
                h_sb[:, z * 2 * R : (z + 1) * 2 * R],
            )

        # expansion: out[z*64+o', u] = relu(h2[z*64+o', pr*65+p+u//32]*T[u] + bias)
        for pr in range(2):
            for p in range(2):
                off = pr * R + p
                h_exp = h2_sb[:, off : off + K].unsqueeze(2).to_broadcast(
                    (128, K, NK)
                )
                tmp = tmp_p.tile([128, Q], F32, tag="tmp")
                nc.vector.tensor_tensor(
                    tmp.rearrange("a (m l) -> a m l", l=NK),
                    h_exp,
                    T_sb.rearrange("a (m l) -> a m l", l=NK),
                    mybir.AluOpType.mult,
                )
                o_sb = sbuf_o.tile([128, Q], F32, tag="o_sb")
                if p == 0:
                    nc.scalar.activation(
                        o_sb[:], tmp[:], mybir.ActivationFunctionType.Relu,
                        bias=b_sb[:],
                    )
                else:
                    nc.vector.tensor_scalar(
                        o_sb[:], tmp[:], b_sb[:], 0.0,
                        mybir.AluOpType.add, mybir.AluOpType.max,
                    )
                nc.sync.dma_start(
                    out[pr * 128 : (pr + 1) * 128, p * Q : (p + 1) * Q], o_sb[:]
                )


def _build_program():
    nc = bacc.Bacc(
        "TRN2", target_bir_lowering=False, debug=False, num_devices=NCORES
    )
    ins = {}
    ins["wbuf"] = nc.dram_tensor(
        "wbuf", [128, NCH * CHUNK_W], BF16, kind="ExternalInput"
    ).ap()
    ins["trow"] = nc.dram_tensor("trow", [1, Q], F32, kind="ExternalInput").ap()
    ins["bias2"] = nc.dram_tensor("bias2", [128, 1], F32, kind="ExternalInput").ap()
    out = nc.dram_tensor("out", [2 * 128, S], F32, kind="ExternalOutput").ap()

    with tile.TileContext(nc) as tc:
        _kernel_body(tc, out, ins)
    nc.compile()
    return nc


def _split_bf16(a):
    hi = a.astype(ml_dtypes.bfloat16)
    lo = (a - hi.astype(np.float32)).astype(ml_dtypes.bfloat16)
    return hi, lo


def _host_prepare(x, stft_w, conv_w, conv_b):
    """Build per-core input maps."""
    x = np.ascontiguousarray(x, dtype=np.float32)
    xp = np.zeros((B, C, 2 * Q + K), dtype=np.float32)  # padded to 4160
    xp[:, :, :S] = x
    sb_, sc_, ss_ = xp.strides
    win = np.lib.stride_tricks.as_strided(
        xp, shape=(B, C, Q, R), strides=(sb_, sc_, 2 * ss_, ss_)
    )
    Xf = win.reshape(B, C * Q, R)                      # (4, 4096, 65), b=2*pr+z
    # layout [p, kt, z, pr, r]: batch order (z,pr) -> b = [0, 2, 1, 3]
    X5 = np.ascontiguousarray(
        Xf[[0, 2, 1, 3]].reshape(2, 2, KT, 128, R).transpose(3, 2, 0, 1, 4)
    ).reshape(128, KT, W260)
    xh, xl = _split_bf16(X5)

    trow = np.ascontiguousarray(stft_w.T, dtype=np.float32).reshape(1, Q)

    in_maps = []
    for i in range(NCORES):
        cw_sh = conv_w[i * OSH : (i + 1) * OSH, :]     # (64, 4096)
        cwt = np.ascontiguousarray(
            cw_sh.reshape(OSH, KT, 128).transpose(2, 1, 0)  # (128, 32, 64)
        )
        ch, cl = _split_bf16(cwt)
        # merged chunk buffer: per chunk [ch | cl | xh | xl], kt-major inside
        parts = [ch, cl, xh, xl] if PASSES == 3 else [ch, xh]
        wbuf = np.empty((128, NCH, NPASS_COLS * KTC), dtype=ml_dtypes.bfloat16)
        for chk in range(NCH):
            sl = slice(chk * KTC, (chk + 1) * KTC)
            wbuf[:, chk, :] = np.concatenate(
                [p_[:, sl].reshape(128, -1) for p_ in parts], axis=1
            )
        bias2 = np.ascontiguousarray(
            np.tile(conv_b[i * OSH : (i + 1) * OSH], 2).reshape(128, 1),
            dtype=np.float32,
        )
        in_maps.append(
            {
                "wbuf": wbuf.reshape(128, NCH * CHUNK_W),
                "trow": trow,
                "bias2": bias2,
            }
        )
    return in_maps


def kernel(x, stft_w, conv_w, conv_b):
    global _PROGRAM, _LAST_RESULTS
    if _PROGRAM is None:
        _PROGRAM = _build_program()
    in_maps = _host_prepare(
        np.asarray(x), np.asarray(stft_w, dtype=np.float32),
        np.asarray(conv_w, dtype=np.float32), np.asarray(conv_b, dtype=np.float32),
    )
    res = run_bass_kernel_spmd(_PROGRAM, in_maps, list(range(NCORES)))
    _LAST_RESULTS = res
    # per-core out: (256, 4096); rows pr*128 + z*64 + o' -> (b=2*pr+z, o=i*64+o')
    full = np.empty((B, OUT, S), dtype=np.float32)
    for i in range(NCORES):
        full[:, i * OSH : (i + 1) * OSH, :] = res.results[i]["out"].reshape(
            B, OSH, S
        )
    return full


if __name__ == "__main__":
    rng = np.random.default_rng(0)
    out = kernel(
        rng.standard_normal((B, C, S), dtype=np.float32),
        rng.standard_normal((NK, K), dtype=np.float32),
        (rng.standard_normal((OUT, C * K * NK)) * 0.02).astype(np.float32),
        (rng.standard_normal((OUT,)) * 0.02).astype(np.float32),
    )
    print(out.shape, out.dtype, float(np.abs(out).max()))
